# revision 12
# baseline (speedup 1.0000x reference)
"""CascadedBranch (retrieval_knn) Trainium2 kernel.

Reference computation (B=256, K=8, Da=768, Dt=512, V=49408):
    proj = audio_kw @ W_proj + b_proj          # [B,K,Dt]
    bn   = batchnorm over (B,K) with gamma/beta
    cos  = normalize(bn) @ normalize(emb).T    # [B,K,V]
    prob = softmax(cos / 0.1)
    out  = prob @ emb                          # [B,K,Dt]

Strategy: shard the vocab axis V across the 8 cores (6272 rows each after
padding 49408 -> 50176). Each core computes, for all 2048 rows:
    projT (W.T @ audio.T, PSUM f32), batchnorm stats via bn_stats/bn_aggr,
    normalized bnT (all in [d, row] layout so BN params are per-partition),
    scores sT[v,row] = embT_n @ bnT_n, expt = exp(10*sT) (bf16),
    u = sum_v expt*emb (PE, accumulated over v-blocks in PSUM),
    s = sum_v expt (DVE accumulate + ones-matmul partition reduce).
No max-subtraction is needed: |cos|<=~1 so logits are in [-10.2, 10.2].
Host combines: out = sum_c u_c / (sum_c s_c - 768)  (the 768 zero pad rows
of core 7 contribute exactly exp(0)=1 to s and 0 to u).
b_proj is ignored: a linear bias cancels exactly inside batchnorm.

Everything needed is hardcoded; no sibling imports.
"""

import sys
import types

import numpy as np
import ml_dtypes

import concourse.bass as bass
import concourse.bacc as bacc
import concourse.tile as tile
from concourse import mybir
from concourse import bass_isa
from concourse.bass_utils import run_bass_kernel_spmd

F32 = mybir.dt.float32
BF16 = mybir.dt.bfloat16

B, K, DA, D, V = 256, 8, 768, 512, 49408
R = B * K              # 2048 rows
NCORES = 8
VS = 6272              # per-core vocab shard (padded)
NVB = VS // 128        # 49 v-blocks
NRC = 4                # row chunks of 512
RC = 512
NDC = D // 128         # 4 d-chunks
NKC = DA // 128        # 6 k-chunks
NPAD = VS * NCORES - V  # 768 zero pad rows (all in core 7's shard)
VQ_TEMP = 0.1
BN_EPS = 1e-5


def _split_sync_waits(nc):
    """The walrus in this image rejects >1 sem-wait per instruction
    ("Too many sync wait commands"). Legalize by inserting single-wait
    Drain carriers immediately before any multi-wait instruction (same
    engine, same basic block position => identical synchronization)."""
    import orjson
    js = orjson.loads(mybir.module_to_json_bytes(nc.m))
    ctr = 0
    for func in js["functions"]:
        for bb in func["blocks"]:
            out = []
            changed = False
            for inst in bb["instructions"]:
                si = inst.get("sync_info")
                waits = (si or {}).get("on_wait") or []
                if len(waits) > 1:
                    changed = True
                    for w in waits[:-1]:
                        ctr += 1
                        carrier = {
                            "name": f"I-lsw-{ctr}",
                            "opcode": "Drain",
                            "engine": inst["engine"],
                            "ins": [],
                            "outs": [],
                            "sync_info": {"on_wait": [w], "on_update": []},
                        }
                        if "debug" in inst:
                            carrier["debug"] = inst["debug"]
                        out.append(carrier)
                    si["on_wait"] = [waits[-1]]
                out.append(inst)
            if changed:
                bb["instructions"] = out
    nc.m = mybir.module_from_json_bytes(orjson.dumps(js))
    return nc


def _patch_upload_artifacts():
    import concourse.bass_utils as bu
    bu.upload_artifacts = lambda tmpdir: "local://" + str(tmpdir)


def _build_kernel():
    nc = bacc.Bacc("TRN2", target_bir_lowering=False)

    # inputs, host-prepped into [128, ...] partition-major layouts
    audio_d = nc.dram_tensor("audioTb", [128, NKC, R], BF16, kind="ExternalInput")
    w_d = nc.dram_tensor("wb", [128, NKC, D], BF16, kind="ExternalInput")
    gamma_d = nc.dram_tensor("gammab", [128, NDC], F32, kind="ExternalInput")
    beta_d = nc.dram_tensor("betab", [128, NDC], F32, kind="ExternalInput")
    embtn_d = nc.dram_tensor("embTnb", [128, NDC, VS], BF16, kind="ExternalInput")
    emb_d = nc.dram_tensor("embb", [128, NVB, D], BF16, kind="ExternalInput")
    u_d = nc.dram_tensor("u", [R, D], F32, kind="ExternalOutput")
    s_d = nc.dram_tensor("s", [NRC, RC], F32, kind="ExternalOutput")

    with tile.TileContext(nc) as tc:
        with (
            tc.tile_pool(name="consts", bufs=1) as consts,
            tc.tile_pool(name="persist", bufs=1) as persist,
            tc.tile_pool(name="sqp", bufs=5) as sqp,
            tc.tile_pool(name="sqsp", bufs=2) as sqsp,
            tc.tile_pool(name="rbcp", bufs=2) as rbcp,
            tc.tile_pool(name="expp", bufs=3) as expp,
            tc.tile_pool(name="accp", bufs=2) as accp,
            tc.tile_pool(name="outp", bufs=3) as outp,
            tc.tile_pool(name="psA", bufs=3, space="PSUM") as psA,
            tc.tile_pool(name="psB", bufs=4, space="PSUM") as psB,
        ):
            # ---- load inputs ----
            # per-k-chunk DMAs so the proj GEMM can start on chunk 0 early
            audio_sb = consts.tile([128, NKC, R], BF16, tag="audio")
            w_sb = consts.tile([128, NKC, D], BF16, tag="w")
            for a in range(NKC):
                nc.gpsimd.dma_start(out=w_sb[:, a, :], in_=w_d[:, a, :])
                nc.gpsimd.dma_start(out=audio_sb[:, a, :], in_=audio_d[:, a, :])
            gamma_sb = consts.tile([128, NDC], F32, tag="gamma")
            nc.gpsimd.dma_start(out=gamma_sb[:, :], in_=gamma_d[:, :])
            beta_sb = consts.tile([128, NDC], F32, tag="beta")
            nc.gpsimd.dma_start(out=beta_sb[:, :], in_=beta_d[:, :])
            embtn_sb = consts.tile([128, NDC, VS], BF16, tag="embtn")
            nc.gpsimd.dma_start(out=embtn_sb[:, :, :], in_=embtn_d[:, :, :])
            emb_sb = consts.tile([128, NVB, D], BF16, tag="emb")
            nc.gpsimd.dma_start(out=emb_sb[:, :, :], in_=emb_d[:, :, :])

            eps_sb = consts.tile([128, 1], F32, tag="eps")
            nc.vector.memset(eps_sb, BN_EPS)

            # ---- phase B: projT = W.T @ audio.T, f32 psum; stats ----
            projT = [persist.tile([128, R], BF16, tag=f"projT{dc}", name=f"projT{dc}") for dc in range(NDC)]
            stats = [persist.tile([128, NRC, 6], F32, tag=f"stats{dc}", name=f"stats{dc}") for dc in range(NDC)]
            for rc in range(NRC):
                rs = slice(rc * RC, (rc + 1) * RC)
                for dc in range(NDC):
                    ps = psA.tile([128, RC], F32, tag="psA")
                    for a in range(NKC):
                        nc.tensor.matmul(
                            ps[:, :],
                            w_sb[:, a, dc * 128:(dc + 1) * 128],
                            audio_sb[:, a, rs],
                            start=(a == 0),
                            stop=(a == NKC - 1),
                        )
                    nc.vector.bn_stats(out=stats[dc][:, rc, :], in_=ps[:, :])
                    nc.vector.tensor_copy(projT[dc][:, rs], ps[:, :])

            # ---- phase C: finalize BN affine params per d-chunk ----
            sdc, bdc = [], []
            for dc in range(NDC):
                mv = persist.tile([128, 2], F32, tag=f"mv{dc}")
                nc.vector.bn_aggr(out=mv[:, :], in_=stats[dc][:, :, :])
                std = persist.tile([128, 1], F32, tag=f"std{dc}")
                nc.scalar.activation(
                    out=std[:, :], in_=mv[:, 1:2],
                    func=mybir.ActivationFunctionType.Sqrt,
                    bias=eps_sb[:, 0:1], scale=1.0,
                )
                rstd = persist.tile([128, 1], F32, tag=f"rstd{dc}")
                nc.vector.reciprocal(out=rstd[:, :], in_=std[:, :])
                s_aff = persist.tile([128, 1], F32, tag=f"saff{dc}")
                nc.vector.tensor_mul(s_aff[:, :], rstd[:, :], gamma_sb[:, dc:dc + 1])
                tmp = persist.tile([128, 1], F32, tag=f"tmp{dc}")
                nc.vector.tensor_mul(tmp[:, :], mv[:, 0:1], s_aff[:, :])
                b_aff = persist.tile([128, 1], F32, tag=f"baff{dc}")
                nc.vector.tensor_tensor(
                    out=b_aff[:, :], in0=beta_sb[:, dc:dc + 1], in1=tmp[:, :],
                    op=mybir.AluOpType.subtract,
                )
                sdc.append(s_aff)
                bdc.append(b_aff)

            # ---- phase D: bn affine (into bnnT in place), row norms, normalize ----
            # row-norm reduce over partitions + broadcast both via gpsimd
            # partition_all_reduce (replicated output), keeping PE free.
            bnnT = [persist.tile([128, R], BF16, tag=f"bnnT{dc}", name=f"bnnT{dc}") for dc in range(NDC)]
            for rc in range(NRC):
                rs = slice(rc * RC, (rc + 1) * RC)
                sqs = []
                for dc in range(NDC):
                    nc.vector.tensor_scalar(
                        out=bnnT[dc][:, rs], in0=projT[dc][:, rs],
                        scalar1=sdc[dc][:, 0:1], scalar2=bdc[dc][:, 0:1],
                        op0=mybir.AluOpType.mult, op1=mybir.AluOpType.add,
                    )
                    sq = sqp.tile([128, RC], BF16, tag="sq", name=f"sq{rc}_{dc}")
                    nc.vector.tensor_mul(sq[:, :], bnnT[dc][:, rs], bnnT[dc][:, rs])
                    sqs.append(sq)
                sqsum = sqsp.tile([128, RC], F32, tag="sqsum")
                nc.vector.tensor_add(sqsum[:, :], sqs[0][:, :], sqs[1][:, :])
                nc.vector.tensor_add(sqsum[:, :], sqsum[:, :], sqs[2][:, :])
                nc.vector.tensor_add(sqsum[:, :], sqsum[:, :], sqs[3][:, :])
                rbc = rbcp.tile([128, RC], F32, tag="rbc")
                nc.gpsimd.partition_all_reduce(
                    rbc[:, :], sqsum[:, :], channels=128,
                    reduce_op=bass_isa.ReduceOp.add,
                )
                nc.scalar.activation(
                    out=rbc[:, :], in_=rbc[:, :],
                    func=mybir.ActivationFunctionType.Sqrt,
                )
                nc.vector.reciprocal(out=rbc[:, :], in_=rbc[:, :])
                for dc in range(NDC):
                    nc.vector.tensor_mul(bnnT[dc][:, rs], bnnT[dc][:, rs], rbc[:, :])

            # ---- phase E: scores -> exp -> u, s ----
            for rc in range(NRC):
                rs = slice(rc * RC, (rc + 1) * RC)
                sumacc = accp.tile([128, RC], F32, tag="sumacc")
                nc.vector.memset(sumacc, 0.0)
                psu = [psB.tile([128, D], F32, tag="psB", name=f"psu{rc}_{i}") for i in range(4)]
                for vb in range(NVB):
                    ps = psA.tile([128, RC], F32, tag="psA")
                    for dc in range(NDC):
                        nc.tensor.matmul(
                            ps[:, :],
                            embtn_sb[:, dc, vb * 128:(vb + 1) * 128],
                            bnnT[dc][:, rs],
                            start=(dc == 0), stop=(dc == NDC - 1),
                        )
                    expt = expp.tile([128, RC], BF16, tag="expt")
                    nc.scalar.activation(
                        out=expt[:, :], in_=ps[:, :],
                        func=mybir.ActivationFunctionType.Exp,
                        scale=1.0 / VQ_TEMP,
                    )
                    nc.vector.tensor_add(sumacc[:, :], sumacc[:, :], expt[:, :])
                    for rsub in range(4):
                        nc.tensor.matmul(
                            psu[rsub][:, :],
                            expt[:, rsub * 128:(rsub + 1) * 128],
                            emb_sb[:, vb, :],
                            start=(vb == 0), stop=(vb == NVB - 1),
                        )
                # s[rc, :] = partition-reduce of sumacc (gpsimd, off PE)
                spar = rbcp.tile([128, RC], F32, tag="spar")
                nc.gpsimd.partition_all_reduce(
                    spar[:, :], sumacc[:, :], channels=128,
                    reduce_op=bass_isa.ReduceOp.add,
                )
                nc.gpsimd.dma_start(out=s_d[rc:rc + 1, :], in_=spar[0:1, :])
                for rsub in range(4):
                    ur = outp.tile([128, D], F32, tag="ur")
                    nc.vector.tensor_copy(ur[:, :], psu[rsub][:, :])
                    r0 = (rc * 4 + rsub) * 128
                    nc.gpsimd.dma_start(out=u_d[r0:r0 + 128, :], in_=ur[:, :])

    nc.compile()
    _split_sync_waits(nc)
    return nc


_NC = None


def kernel(audio_kw, W_proj, b_proj, bn_gamma, bn_beta, emb):
    global _NC
    audio_kw = np.asarray(audio_kw, dtype=np.float32)
    W_proj = np.asarray(W_proj, dtype=np.float32)
    bn_gamma = np.asarray(bn_gamma, dtype=np.float32)
    bn_beta = np.asarray(bn_beta, dtype=np.float32)
    emb = np.asarray(emb, dtype=np.float32)

    # host prep: partition-major device layouts
    audioT = np.ascontiguousarray(
        audio_kw.reshape(R, DA).T.reshape(NKC, 128, R).transpose(1, 0, 2)
    ).astype(ml_dtypes.bfloat16)
    wb = np.ascontiguousarray(
        W_proj.reshape(NKC, 128, D).transpose(1, 0, 2)
    ).astype(ml_dtypes.bfloat16)
    gammab = np.ascontiguousarray(bn_gamma.reshape(NDC, 128).T)
    betab = np.ascontiguousarray(bn_beta.reshape(NDC, 128).T)

    norms = np.linalg.norm(emb, axis=1, keepdims=True)
    emb_n = emb / norms
    vtot = VS * NCORES
    embTn_pad = np.zeros((D, vtot), dtype=np.float32)
    embTn_pad[:, :V] = emb_n.T
    emb_pad = np.zeros((vtot, D), dtype=np.float32)
    emb_pad[:V] = emb

    in_maps = []
    for c in range(NCORES):
        etn = np.ascontiguousarray(
            embTn_pad[:, c * VS:(c + 1) * VS]
            .reshape(NDC, 128, VS).transpose(1, 0, 2)
        ).astype(ml_dtypes.bfloat16)
        eb = np.ascontiguousarray(
            emb_pad[c * VS:(c + 1) * VS]
            .reshape(NVB, 128, D).transpose(1, 0, 2)
        ).astype(ml_dtypes.bfloat16)
        in_maps.append({
            "audioTb": audioT, "wb": wb, "gammab": gammab, "betab": betab,
            "embTnb": etn, "embb": eb,
        })

    if _NC is None:
        _NC = _build_kernel()
    _patch_upload_artifacts()
    res = run_bass_kernel_spmd(_NC, in_maps, core_ids=list(range(NCORES)))

    u_tot = np.zeros((R, D), dtype=np.float64)
    s_tot = np.zeros((R,), dtype=np.float64)
    for c in range(NCORES):
        u_tot += res.results[c]["u"].astype(np.float64)
        s_tot += res.results[c]["s"].reshape(R).astype(np.float64)
    s_tot -= NPAD  # zero pad rows contribute exactly exp(0)=1 each
    out = (u_tot / s_tot[:, None]).astype(np.float32)
    return out.reshape(B, K, D)


# revision 13
# speedup vs baseline: 1.0181x; 1.0181x over previous
"""CascadedBranch (retrieval_knn) Trainium2 kernel.

Reference computation (B=256, K=8, Da=768, Dt=512, V=49408):
    proj = audio_kw @ W_proj + b_proj          # [B,K,Dt]
    bn   = batchnorm over (B,K) with gamma/beta
    cos  = normalize(bn) @ normalize(emb).T    # [B,K,V]
    prob = softmax(cos / 0.1)
    out  = prob @ emb                          # [B,K,Dt]

Strategy: shard the vocab axis V across the 8 cores (6272 rows each after
padding 49408 -> 50176). Each core computes, for all 2048 rows:
    projT (W.T @ audio.T, PSUM f32), batchnorm stats via bn_stats/bn_aggr,
    normalized bnT (all in [d, row] layout so BN params are per-partition),
    scores sT[v,row] = embT_n @ bnT_n, expt = exp(10*sT) (bf16),
    u = sum_v expt*emb (PE, accumulated over v-blocks in PSUM),
    s = sum_v expt (DVE accumulate + ones-matmul partition reduce).
No max-subtraction is needed: |cos|<=~1 so logits are in [-10.2, 10.2].
Host combines: out = sum_c u_c / (sum_c s_c - 768)  (the 768 zero pad rows
of core 7 contribute exactly exp(0)=1 to s and 0 to u).
b_proj is ignored: a linear bias cancels exactly inside batchnorm.

Everything needed is hardcoded; no sibling imports.
"""

import sys
import types

import numpy as np
import ml_dtypes

import concourse.bass as bass
import concourse.bacc as bacc
import concourse.tile as tile
from concourse import mybir
from concourse import bass_isa
from concourse.bass_utils import run_bass_kernel_spmd

F32 = mybir.dt.float32
BF16 = mybir.dt.bfloat16

B, K, DA, D, V = 256, 8, 768, 512, 49408
R = B * K              # 2048 rows
NCORES = 8
VS = 6272              # per-core vocab shard (padded)
NVB = VS // 128        # 49 v-blocks
NRC = 4                # row chunks of 512
RC = 512
NDC = D // 128         # 4 d-chunks
NKC = DA // 128        # 6 k-chunks
NPAD = VS * NCORES - V  # 768 zero pad rows (all in core 7's shard)
VQ_TEMP = 0.1
BN_EPS = 1e-5


def _split_sync_waits(nc):
    """The walrus in this image rejects >1 sem-wait per instruction
    ("Too many sync wait commands"). Legalize by inserting single-wait
    Drain carriers immediately before any multi-wait instruction (same
    engine, same basic block position => identical synchronization)."""
    import orjson
    js = orjson.loads(mybir.module_to_json_bytes(nc.m))
    ctr = 0
    for func in js["functions"]:
        for bb in func["blocks"]:
            out = []
            changed = False
            for inst in bb["instructions"]:
                si = inst.get("sync_info")
                waits = (si or {}).get("on_wait") or []
                if len(waits) > 1:
                    changed = True
                    for w in waits[:-1]:
                        ctr += 1
                        carrier = {
                            "name": f"I-lsw-{ctr}",
                            "opcode": "Drain",
                            "engine": inst["engine"],
                            "ins": [],
                            "outs": [],
                            "sync_info": {"on_wait": [w], "on_update": []},
                        }
                        if "debug" in inst:
                            carrier["debug"] = inst["debug"]
                        out.append(carrier)
                    si["on_wait"] = [waits[-1]]
                out.append(inst)
            if changed:
                bb["instructions"] = out
    nc.m = mybir.module_from_json_bytes(orjson.dumps(js))
    return nc


def _patch_upload_artifacts():
    import concourse.bass_utils as bu
    bu.upload_artifacts = lambda tmpdir: "local://" + str(tmpdir)


def _build_kernel():
    nc = bacc.Bacc("TRN2", target_bir_lowering=False)

    # inputs, host-prepped into [128, ...] partition-major layouts
    audio_d = nc.dram_tensor("audioTb", [128, NKC, R], BF16, kind="ExternalInput")
    w_d = nc.dram_tensor("wb", [128, NKC, D], BF16, kind="ExternalInput")
    gamma_d = nc.dram_tensor("gammab", [128, NDC], F32, kind="ExternalInput")
    beta_d = nc.dram_tensor("betab", [128, NDC], F32, kind="ExternalInput")
    embtn_d = nc.dram_tensor("embTnb", [128, NDC, VS], BF16, kind="ExternalInput")
    emb_d = nc.dram_tensor("embb", [128, NVB, D], BF16, kind="ExternalInput")
    u_d = nc.dram_tensor("u", [R, D], F32, kind="ExternalOutput")
    s_d = nc.dram_tensor("s", [NRC, RC], F32, kind="ExternalOutput")

    with tile.TileContext(nc) as tc:
        with (
            tc.tile_pool(name="consts", bufs=1) as consts,
            tc.tile_pool(name="persist", bufs=1) as persist,
            tc.tile_pool(name="sqp", bufs=5) as sqp,
            tc.tile_pool(name="sqsp", bufs=2) as sqsp,
            tc.tile_pool(name="rbcp", bufs=3) as rbcp,
            tc.tile_pool(name="expp", bufs=3) as expp,
            tc.tile_pool(name="accp", bufs=4) as accp,
            tc.tile_pool(name="outp", bufs=3) as outp,
            tc.tile_pool(name="psA", bufs=3, space="PSUM") as psA,
            tc.tile_pool(name="psB", bufs=4, space="PSUM") as psB,
        ):
            # ---- load inputs ----
            # per-k-chunk DMAs so the proj GEMM can start on chunk 0 early
            audio_sb = consts.tile([128, NKC, R], BF16, tag="audio")
            w_sb = consts.tile([128, NKC, D], BF16, tag="w")
            for a in range(NKC):
                nc.sync.dma_start(out=w_sb[:, a, :], in_=w_d[:, a, :])
                nc.sync.dma_start(out=audio_sb[:, a, :], in_=audio_d[:, a, :])
            gamma_sb = consts.tile([128, NDC], F32, tag="gamma")
            nc.sync.dma_start(out=gamma_sb[:, :], in_=gamma_d[:, :])
            beta_sb = consts.tile([128, NDC], F32, tag="beta")
            nc.sync.dma_start(out=beta_sb[:, :], in_=beta_d[:, :])
            embtn_sb = consts.tile([128, NDC, VS], BF16, tag="embtn")
            nc.sync.dma_start(out=embtn_sb[:, :, :], in_=embtn_d[:, :, :])
            emb_sb = consts.tile([128, NVB, D], BF16, tag="emb")
            nc.sync.dma_start(out=emb_sb[:, :, :], in_=emb_d[:, :, :])

            eps_sb = consts.tile([128, 1], F32, tag="eps")
            nc.vector.memset(eps_sb, BN_EPS)

            # ---- phase B: projT = W.T @ audio.T, f32 psum; stats ----
            projT = [persist.tile([128, R], BF16, tag=f"projT{dc}", name=f"projT{dc}") for dc in range(NDC)]
            stats = [persist.tile([128, NRC, 6], F32, tag=f"stats{dc}", name=f"stats{dc}") for dc in range(NDC)]
            for rc in range(NRC):
                rs = slice(rc * RC, (rc + 1) * RC)
                for dc in range(NDC):
                    ps = psA.tile([128, RC], F32, tag="psA")
                    for a in range(NKC):
                        nc.tensor.matmul(
                            ps[:, :],
                            w_sb[:, a, dc * 128:(dc + 1) * 128],
                            audio_sb[:, a, rs],
                            start=(a == 0),
                            stop=(a == NKC - 1),
                        )
                    nc.vector.bn_stats(out=stats[dc][:, rc, :], in_=ps[:, :])
                    nc.vector.tensor_copy(projT[dc][:, rs], ps[:, :])

            # ---- phase C: finalize BN affine params per d-chunk ----
            sdc, bdc = [], []
            for dc in range(NDC):
                mv = persist.tile([128, 2], F32, tag=f"mv{dc}")
                nc.vector.bn_aggr(out=mv[:, :], in_=stats[dc][:, :, :])
                std = persist.tile([128, 1], F32, tag=f"std{dc}")
                nc.scalar.activation(
                    out=std[:, :], in_=mv[:, 1:2],
                    func=mybir.ActivationFunctionType.Sqrt,
                    bias=eps_sb[:, 0:1], scale=1.0,
                )
                rstd = persist.tile([128, 1], F32, tag=f"rstd{dc}")
                nc.vector.reciprocal(out=rstd[:, :], in_=std[:, :])
                s_aff = persist.tile([128, 1], F32, tag=f"saff{dc}")
                nc.vector.tensor_mul(s_aff[:, :], rstd[:, :], gamma_sb[:, dc:dc + 1])
                tmp = persist.tile([128, 1], F32, tag=f"tmp{dc}")
                nc.vector.tensor_mul(tmp[:, :], mv[:, 0:1], s_aff[:, :])
                b_aff = persist.tile([128, 1], F32, tag=f"baff{dc}")
                nc.vector.tensor_tensor(
                    out=b_aff[:, :], in0=beta_sb[:, dc:dc + 1], in1=tmp[:, :],
                    op=mybir.AluOpType.subtract,
                )
                sdc.append(s_aff)
                bdc.append(b_aff)

            # ---- phase D: bn affine (into bnnT in place), row norms, normalize ----
            # row-norm reduce over partitions + broadcast both via gpsimd
            # partition_all_reduce (replicated output), keeping PE free.
            bnnT = [persist.tile([128, R], BF16, tag=f"bnnT{dc}", name=f"bnnT{dc}") for dc in range(NDC)]
            for rc in range(NRC):
                rs = slice(rc * RC, (rc + 1) * RC)
                sqs = []
                for dc in range(NDC):
                    nc.vector.tensor_scalar(
                        out=bnnT[dc][:, rs], in0=projT[dc][:, rs],
                        scalar1=sdc[dc][:, 0:1], scalar2=bdc[dc][:, 0:1],
                        op0=mybir.AluOpType.mult, op1=mybir.AluOpType.add,
                    )
                    sq = sqp.tile([128, RC], BF16, tag="sq", name=f"sq{rc}_{dc}")
                    nc.vector.tensor_mul(sq[:, :], bnnT[dc][:, rs], bnnT[dc][:, rs])
                    sqs.append(sq)
                sqsum = sqsp.tile([128, RC], F32, tag="sqsum")
                nc.vector.tensor_add(sqsum[:, :], sqs[0][:, :], sqs[1][:, :])
                nc.vector.tensor_add(sqsum[:, :], sqsum[:, :], sqs[2][:, :])
                nc.vector.tensor_add(sqsum[:, :], sqsum[:, :], sqs[3][:, :])
                rbc = rbcp.tile([128, RC], F32, tag="rbc")
                nc.gpsimd.partition_all_reduce(
                    rbc[:, :], sqsum[:, :], channels=128,
                    reduce_op=bass_isa.ReduceOp.add,
                )
                nc.scalar.activation(
                    out=rbc[:, :], in_=rbc[:, :],
                    func=mybir.ActivationFunctionType.Sqrt,
                )
                nc.vector.reciprocal(out=rbc[:, :], in_=rbc[:, :])
                for dc in range(NDC):
                    nc.vector.tensor_mul(bnnT[dc][:, rs], bnnT[dc][:, rs], rbc[:, :])

            # ---- phase E: scores -> exp -> u, s ----
            for rc in range(NRC):
                rs = slice(rc * RC, (rc + 1) * RC)
                sumacc = accp.tile([128, RC], F32, tag="sumacc")
                nc.vector.memset(sumacc, 0.0)
                psu = [psB.tile([128, D], F32, tag="psB", name=f"psu{rc}_{i}") for i in range(4)]
                for vb in range(NVB):
                    ps = psA.tile([128, RC], F32, tag="psA")
                    for dc in range(NDC):
                        nc.tensor.matmul(
                            ps[:, :],
                            embtn_sb[:, dc, vb * 128:(vb + 1) * 128],
                            bnnT[dc][:, rs],
                            start=(dc == 0), stop=(dc == NDC - 1),
                        )
                    expt = expp.tile([128, RC], BF16, tag="expt")
                    nc.scalar.activation(
                        out=expt[:, :], in_=ps[:, :],
                        func=mybir.ActivationFunctionType.Exp,
                        scale=1.0 / VQ_TEMP,
                    )
                    nc.vector.tensor_add(sumacc[:, :], sumacc[:, :], expt[:, :])
                    for rsub in range(4):
                        nc.tensor.matmul(
                            psu[rsub][:, :],
                            expt[:, rsub * 128:(rsub + 1) * 128],
                            emb_sb[:, vb, :],
                            start=(vb == 0), stop=(vb == NVB - 1),
                        )
                # s[rc, :] = partition-reduce of sumacc (gpsimd, off PE)
                spar = rbcp.tile([128, RC], F32, tag="spar")
                nc.gpsimd.partition_all_reduce(
                    spar[:, :], sumacc[:, :], channels=128,
                    reduce_op=bass_isa.ReduceOp.add,
                )
                nc.sync.dma_start(out=s_d[rc:rc + 1, :], in_=spar[0:1, :])
                for rsub in range(4):
                    ur = outp.tile([128, D], F32, tag="ur")
                    nc.vector.tensor_copy(ur[:, :], psu[rsub][:, :])
                    r0 = (rc * 4 + rsub) * 128
                    nc.sync.dma_start(out=u_d[r0:r0 + 128, :], in_=ur[:, :])

    nc.compile()
    _split_sync_waits(nc)
    return nc


_NC = None


def kernel(audio_kw, W_proj, b_proj, bn_gamma, bn_beta, emb):
    global _NC
    audio_kw = np.asarray(audio_kw, dtype=np.float32)
    W_proj = np.asarray(W_proj, dtype=np.float32)
    bn_gamma = np.asarray(bn_gamma, dtype=np.float32)
    bn_beta = np.asarray(bn_beta, dtype=np.float32)
    emb = np.asarray(emb, dtype=np.float32)

    # host prep: partition-major device layouts
    audioT = np.ascontiguousarray(
        audio_kw.reshape(R, DA).T.reshape(NKC, 128, R).transpose(1, 0, 2)
    ).astype(ml_dtypes.bfloat16)
    wb = np.ascontiguousarray(
        W_proj.reshape(NKC, 128, D).transpose(1, 0, 2)
    ).astype(ml_dtypes.bfloat16)
    gammab = np.ascontiguousarray(bn_gamma.reshape(NDC, 128).T)
    betab = np.ascontiguousarray(bn_beta.reshape(NDC, 128).T)

    norms = np.linalg.norm(emb, axis=1, keepdims=True)
    emb_n = emb / norms
    vtot = VS * NCORES
    embTn_pad = np.zeros((D, vtot), dtype=np.float32)
    embTn_pad[:, :V] = emb_n.T
    emb_pad = np.zeros((vtot, D), dtype=np.float32)
    emb_pad[:V] = emb

    in_maps = []
    for c in range(NCORES):
        etn = np.ascontiguousarray(
            embTn_pad[:, c * VS:(c + 1) * VS]
            .reshape(NDC, 128, VS).transpose(1, 0, 2)
        ).astype(ml_dtypes.bfloat16)
        eb = np.ascontiguousarray(
            emb_pad[c * VS:(c + 1) * VS]
            .reshape(NVB, 128, D).transpose(1, 0, 2)
        ).astype(ml_dtypes.bfloat16)
        in_maps.append({
            "audioTb": audioT, "wb": wb, "gammab": gammab, "betab": betab,
            "embTnb": etn, "embb": eb,
        })

    if _NC is None:
        _NC = _build_kernel()
    _patch_upload_artifacts()
    res = run_bass_kernel_spmd(_NC, in_maps, core_ids=list(range(NCORES)))

    u_tot = np.zeros((R, D), dtype=np.float64)
    s_tot = np.zeros((R,), dtype=np.float64)
    for c in range(NCORES):
        u_tot += res.results[c]["u"].astype(np.float64)
        s_tot += res.results[c]["s"].reshape(R).astype(np.float64)
    s_tot -= NPAD  # zero pad rows contribute exactly exp(0)=1 each
    out = (u_tot / s_tot[:, None]).astype(np.float32)
    return out.reshape(B, K, D)


# revision 15
# speedup vs baseline: 1.0213x; 1.0032x over previous
"""CascadedBranch (retrieval_knn) Trainium2 kernel.

Reference computation (B=256, K=8, Da=768, Dt=512, V=49408):
    proj = audio_kw @ W_proj + b_proj          # [B,K,Dt]
    bn   = batchnorm over (B,K) with gamma/beta
    cos  = normalize(bn) @ normalize(emb).T    # [B,K,V]
    prob = softmax(cos / 0.1)
    out  = prob @ emb                          # [B,K,Dt]

Strategy: shard the vocab axis V across the 8 cores (6272 rows each after
padding 49408 -> 50176). Each core computes, for all 2048 rows:
    projT (W.T @ audio.T, PSUM f32), batchnorm stats via bn_stats/bn_aggr,
    normalized bnT (all in [d, row] layout so BN params are per-partition),
    scores sT[v,row] = embT_n @ bnT_n, expt = exp(10*sT) (bf16),
    u = sum_v expt*emb (PE, accumulated over v-blocks in PSUM),
    s = sum_v expt (DVE accumulate + ones-matmul partition reduce).
No max-subtraction is needed: |cos|<=~1 so logits are in [-10.2, 10.2].
Host combines: out = sum_c u_c / (sum_c s_c - 768)  (the 768 zero pad rows
of core 7 contribute exactly exp(0)=1 to s and 0 to u).
b_proj is ignored: a linear bias cancels exactly inside batchnorm.

Everything needed is hardcoded; no sibling imports.
"""

import sys
import types

import numpy as np
import ml_dtypes

import concourse.bass as bass
import concourse.bacc as bacc
import concourse.tile as tile
from concourse import mybir
from concourse import bass_isa
from concourse.bass_utils import run_bass_kernel_spmd

F32 = mybir.dt.float32
BF16 = mybir.dt.bfloat16

B, K, DA, D, V = 256, 8, 768, 512, 49408
R = B * K              # 2048 rows
NCORES = 8
VS = 6272              # per-core vocab shard (padded)
NVB = VS // 128        # 49 v-blocks
NRC = 4                # row chunks of 512
RC = 512
NDC = D // 128         # 4 d-chunks
NKC = DA // 128        # 6 k-chunks
NPAD = VS * NCORES - V  # 768 zero pad rows (all in core 7's shard)
VQ_TEMP = 0.1
BN_EPS = 1e-5


def _split_sync_waits(nc):
    """The walrus in this image rejects >1 sem-wait per instruction
    ("Too many sync wait commands"). Legalize by inserting single-wait
    Drain carriers immediately before any multi-wait instruction (same
    engine, same basic block position => identical synchronization)."""
    import orjson
    js = orjson.loads(mybir.module_to_json_bytes(nc.m))
    ctr = 0
    for func in js["functions"]:
        for bb in func["blocks"]:
            out = []
            changed = False
            for inst in bb["instructions"]:
                si = inst.get("sync_info")
                waits = (si or {}).get("on_wait") or []
                if len(waits) > 1:
                    changed = True
                    for w in waits[:-1]:
                        ctr += 1
                        carrier = {
                            "name": f"I-lsw-{ctr}",
                            "opcode": "Drain",
                            "engine": inst["engine"],
                            "ins": [],
                            "outs": [],
                            "sync_info": {"on_wait": [w], "on_update": []},
                        }
                        if "debug" in inst:
                            carrier["debug"] = inst["debug"]
                        out.append(carrier)
                    si["on_wait"] = [waits[-1]]
                out.append(inst)
            if changed:
                bb["instructions"] = out
    nc.m = mybir.module_from_json_bytes(orjson.dumps(js))
    return nc


def _patch_upload_artifacts():
    import concourse.bass_utils as bu
    bu.upload_artifacts = lambda tmpdir: "local://" + str(tmpdir)


def _build_kernel():
    nc = bacc.Bacc("TRN2", target_bir_lowering=False)

    # inputs, host-prepped into [128, ...] partition-major layouts
    audio_d = nc.dram_tensor("audioTb", [128, NKC, R], BF16, kind="ExternalInput")
    w_d = nc.dram_tensor("wb", [128, NKC, D], BF16, kind="ExternalInput")
    gamma_d = nc.dram_tensor("gammab", [128, NDC], F32, kind="ExternalInput")
    beta_d = nc.dram_tensor("betab", [128, NDC], F32, kind="ExternalInput")
    embtn_d = nc.dram_tensor("embTnb", [128, NDC, VS], BF16, kind="ExternalInput")
    emb_d = nc.dram_tensor("embb", [128, NVB, D], BF16, kind="ExternalInput")
    u_d = nc.dram_tensor("u", [R, D], F32, kind="ExternalOutput")
    s_d = nc.dram_tensor("s", [NRC, RC], F32, kind="ExternalOutput")

    with tile.TileContext(nc) as tc:
        with (
            tc.tile_pool(name="consts", bufs=1) as consts,
            tc.tile_pool(name="persist", bufs=1) as persist,
            tc.tile_pool(name="sqp", bufs=3) as sqp,
            tc.tile_pool(name="rnp", bufs=2) as rnp,
            tc.tile_pool(name="rbcp", bufs=2) as rbcp,
            tc.tile_pool(name="expp", bufs=4) as expp,
            tc.tile_pool(name="accp", bufs=4) as accp,
            tc.tile_pool(name="outp", bufs=3) as outp,
            tc.tile_pool(name="psA", bufs=2, space="PSUM") as psA,
            tc.tile_pool(name="psB", bufs=4, space="PSUM") as psB,
            tc.tile_pool(name="psC", bufs=1, space="PSUM") as psC,
            tc.tile_pool(name="psD", bufs=1, space="PSUM") as psD,
        ):
            # ---- load inputs ----
            # per-k-chunk DMAs so the proj GEMM can start on chunk 0 early
            audio_sb = consts.tile([128, NKC, R], BF16, tag="audio")
            w_sb = consts.tile([128, NKC, D], BF16, tag="w")
            for a in range(NKC):
                nc.sync.dma_start(out=w_sb[:, a, :], in_=w_d[:, a, :])
                nc.sync.dma_start(out=audio_sb[:, a, :], in_=audio_d[:, a, :])
            gamma_sb = consts.tile([128, NDC], F32, tag="gamma")
            nc.sync.dma_start(out=gamma_sb[:, :], in_=gamma_d[:, :])
            beta_sb = consts.tile([128, NDC], F32, tag="beta")
            nc.sync.dma_start(out=beta_sb[:, :], in_=beta_d[:, :])
            embtn_sb = consts.tile([128, NDC, VS], BF16, tag="embtn")
            nc.sync.dma_start(out=embtn_sb[:, :, :], in_=embtn_d[:, :, :])
            emb_sb = consts.tile([128, NVB, D], BF16, tag="emb")
            nc.sync.dma_start(out=emb_sb[:, :, :], in_=emb_d[:, :, :])

            ones_bf = consts.tile([128, 1], BF16, tag="ones_bf")
            nc.vector.memset(ones_bf, 1.0)
            ones_row = consts.tile([1, 128], F32, tag="ones_row")
            nc.vector.memset(ones_row, 1.0)
            eps_sb = consts.tile([128, 1], F32, tag="eps")
            nc.vector.memset(eps_sb, BN_EPS)

            # ---- phase B: projT = W.T @ audio.T, f32 psum; stats ----
            projT = [persist.tile([128, R], BF16, tag=f"projT{dc}", name=f"projT{dc}") for dc in range(NDC)]
            stats = [persist.tile([128, NRC, 6], F32, tag=f"stats{dc}", name=f"stats{dc}") for dc in range(NDC)]
            for rc in range(NRC):
                rs = slice(rc * RC, (rc + 1) * RC)
                for dc in range(NDC):
                    ps = psA.tile([128, RC], F32, tag="psA")
                    for a in range(NKC):
                        nc.tensor.matmul(
                            ps[:, :],
                            w_sb[:, a, dc * 128:(dc + 1) * 128],
                            audio_sb[:, a, rs],
                            start=(a == 0),
                            stop=(a == NKC - 1),
                        )
                    nc.vector.bn_stats(out=stats[dc][:, rc, :], in_=ps[:, :])
                    nc.vector.tensor_copy(projT[dc][:, rs], ps[:, :])

            # ---- phase C: finalize BN affine params per d-chunk ----
            sdc, bdc = [], []
            for dc in range(NDC):
                mv = persist.tile([128, 2], F32, tag=f"mv{dc}")
                nc.vector.bn_aggr(out=mv[:, :], in_=stats[dc][:, :, :])
                std = persist.tile([128, 1], F32, tag=f"std{dc}")
                nc.scalar.activation(
                    out=std[:, :], in_=mv[:, 1:2],
                    func=mybir.ActivationFunctionType.Sqrt,
                    bias=eps_sb[:, 0:1], scale=1.0,
                )
                rstd = persist.tile([128, 1], F32, tag=f"rstd{dc}")
                nc.vector.reciprocal(out=rstd[:, :], in_=std[:, :])
                s_aff = persist.tile([128, 1], F32, tag=f"saff{dc}")
                nc.vector.tensor_mul(s_aff[:, :], rstd[:, :], gamma_sb[:, dc:dc + 1])
                tmp = persist.tile([128, 1], F32, tag=f"tmp{dc}")
                nc.vector.tensor_mul(tmp[:, :], mv[:, 0:1], s_aff[:, :])
                b_aff = persist.tile([128, 1], F32, tag=f"baff{dc}")
                nc.vector.tensor_tensor(
                    out=b_aff[:, :], in0=beta_sb[:, dc:dc + 1], in1=tmp[:, :],
                    op=mybir.AluOpType.subtract,
                )
                sdc.append(s_aff)
                bdc.append(b_aff)

            # ---- phase D: bn affine (into bnnT in place), row norms, normalize ----
            bnnT = [persist.tile([128, R], BF16, tag=f"bnnT{dc}", name=f"bnnT{dc}") for dc in range(NDC)]
            for rc in range(NRC):
                rs = slice(rc * RC, (rc + 1) * RC)
                norm2 = psC.tile([1, RC], F32, tag="psC", name=f"norm2_{rc}")
                for dc in range(NDC):
                    nc.vector.tensor_scalar(
                        out=bnnT[dc][:, rs], in0=projT[dc][:, rs],
                        scalar1=sdc[dc][:, 0:1], scalar2=bdc[dc][:, 0:1],
                        op0=mybir.AluOpType.mult, op1=mybir.AluOpType.add,
                    )
                    sq = sqp.tile([128, RC], BF16, tag="sq", name=f"sq{rc}_{dc}")
                    nc.vector.tensor_mul(sq[:, :], bnnT[dc][:, rs], bnnT[dc][:, rs])
                    nc.tensor.matmul(
                        norm2[:, :], ones_bf[:, :], sq[:, :],
                        start=(dc == 0), stop=(dc == NDC - 1),
                    )
                rn = rnp.tile([1, RC], F32, tag="rn")
                nc.scalar.activation(
                    out=rn[:, :], in_=norm2[:, :],
                    func=mybir.ActivationFunctionType.Sqrt,
                )
                rninv = rnp.tile([1, RC], F32, tag="rninv")
                nc.vector.reciprocal(out=rninv[:, :], in_=rn[:, :])
                rbc = psD.tile([128, RC], F32, tag="psD")
                nc.tensor.matmul(rbc[:, :], ones_row[:, :], rninv[:, :],
                                 start=True, stop=True)
                for dc in range(NDC):
                    nc.vector.tensor_mul(bnnT[dc][:, rs], bnnT[dc][:, rs], rbc[:, :])

            # ---- phase E: scores -> exp -> u, s ----
            for rc in range(NRC):
                rs = slice(rc * RC, (rc + 1) * RC)
                sumacc = accp.tile([128, RC], F32, tag="sumacc")
                nc.vector.memset(sumacc, 0.0)
                psu = [psB.tile([128, D], F32, tag="psB", name=f"psu{rc}_{i}") for i in range(4)]
                for vb in range(NVB):
                    ps = psA.tile([128, RC], F32, tag="psA")
                    for dc in range(NDC):
                        nc.tensor.matmul(
                            ps[:, :],
                            embtn_sb[:, dc, vb * 128:(vb + 1) * 128],
                            bnnT[dc][:, rs],
                            start=(dc == 0), stop=(dc == NDC - 1),
                        )
                    expt = expp.tile([128, RC], BF16, tag="expt")
                    nc.scalar.activation(
                        out=expt[:, :], in_=ps[:, :],
                        func=mybir.ActivationFunctionType.Exp,
                        scale=1.0 / VQ_TEMP,
                    )
                    nc.vector.tensor_add(sumacc[:, :], sumacc[:, :], expt[:, :])
                    for rsub in range(4):
                        nc.tensor.matmul(
                            psu[rsub][:, :],
                            expt[:, rsub * 128:(rsub + 1) * 128],
                            emb_sb[:, vb, :],
                            start=(vb == 0), stop=(vb == NVB - 1),
                        )
                # s[rc, :] = partition-reduce of sumacc (gpsimd, off PE)
                spar = rbcp.tile([128, RC], F32, tag="spar")
                nc.gpsimd.partition_all_reduce(
                    spar[:, :], sumacc[:, :], channels=128,
                    reduce_op=bass_isa.ReduceOp.add,
                )
                nc.sync.dma_start(out=s_d[rc:rc + 1, :], in_=spar[0:1, :])
                for rsub in range(4):
                    ur = outp.tile([128, D], F32, tag="ur")
                    nc.vector.tensor_copy(ur[:, :], psu[rsub][:, :])
                    r0 = (rc * 4 + rsub) * 128
                    nc.sync.dma_start(out=u_d[r0:r0 + 128, :], in_=ur[:, :])

    nc.compile()
    _split_sync_waits(nc)
    return nc


_NC = None


def kernel(audio_kw, W_proj, b_proj, bn_gamma, bn_beta, emb):
    global _NC
    audio_kw = np.asarray(audio_kw, dtype=np.float32)
    W_proj = np.asarray(W_proj, dtype=np.float32)
    bn_gamma = np.asarray(bn_gamma, dtype=np.float32)
    bn_beta = np.asarray(bn_beta, dtype=np.float32)
    emb = np.asarray(emb, dtype=np.float32)

    # host prep: partition-major device layouts
    audioT = np.ascontiguousarray(
        audio_kw.reshape(R, DA).T.reshape(NKC, 128, R).transpose(1, 0, 2)
    ).astype(ml_dtypes.bfloat16)
    wb = np.ascontiguousarray(
        W_proj.reshape(NKC, 128, D).transpose(1, 0, 2)
    ).astype(ml_dtypes.bfloat16)
    gammab = np.ascontiguousarray(bn_gamma.reshape(NDC, 128).T)
    betab = np.ascontiguousarray(bn_beta.reshape(NDC, 128).T)

    norms = np.linalg.norm(emb, axis=1, keepdims=True)
    emb_n = emb / norms
    vtot = VS * NCORES
    embTn_pad = np.zeros((D, vtot), dtype=np.float32)
    embTn_pad[:, :V] = emb_n.T
    emb_pad = np.zeros((vtot, D), dtype=np.float32)
    emb_pad[:V] = emb

    in_maps = []
    for c in range(NCORES):
        etn = np.ascontiguousarray(
            embTn_pad[:, c * VS:(c + 1) * VS]
            .reshape(NDC, 128, VS).transpose(1, 0, 2)
        ).astype(ml_dtypes.bfloat16)
        eb = np.ascontiguousarray(
            emb_pad[c * VS:(c + 1) * VS]
            .reshape(NVB, 128, D).transpose(1, 0, 2)
        ).astype(ml_dtypes.bfloat16)
        in_maps.append({
            "audioTb": audioT, "wb": wb, "gammab": gammab, "betab": betab,
            "embTnb": etn, "embb": eb,
        })

    if _NC is None:
        _NC = _build_kernel()
    _patch_upload_artifacts()
    res = run_bass_kernel_spmd(_NC, in_maps, core_ids=list(range(NCORES)))

    u_tot = np.zeros((R, D), dtype=np.float64)
    s_tot = np.zeros((R,), dtype=np.float64)
    for c in range(NCORES):
        u_tot += res.results[c]["u"].astype(np.float64)
        s_tot += res.results[c]["s"].reshape(R).astype(np.float64)
    s_tot -= NPAD  # zero pad rows contribute exactly exp(0)=1 each
    out = (u_tot / s_tot[:, None]).astype(np.float32)
    return out.reshape(B, K, D)
